# revision 5
# baseline (speedup 1.0000x reference)
"""Trainium2 Bass kernel for nn_Model_1580547969651.

Math (from the reference):
    s    = x @ sum(y, axis=0)          # (B,) row-sums of x @ y^T
    h    = hardswish(s)                # s * clip(s+3, 0, 6) / 6
    out  = clip(h + noise, -0.5, 0.5)  # (B, 1)

Strategy: COLUMN-shard x and y across the 8 cores (512 features each).
Each core's column-sum of its y shard is locally complete, so there is
no mid-kernel collective: the core reduces y while streaming it (split
between the TensorEngine ones-matmul and VectorEngine adds), broadcasts
its 512-wide ysum across partitions with one K=1 matmul, then computes
partial dot products s_i = x[:, F_i] @ ysum_i for ALL 8192 rows with
fused scalar_tensor_tensor ops while x streams in. A single 32KB->4KB
ReduceScatter at the end sums the partials and hands each core exactly
its 1024-row output shard for the elementwise tail.
"""

import numpy as np

from concourse import bass, bacc, mybir, tile
from concourse.bass_utils import run_bass_kernel_spmd

B = 8192
F = 4096
NCORES = 8
FL = F // NCORES        # 512 features per core
BL = B // NCORES        # 1024 output rows per core
NST = 8                 # y/x super-tiles (128 rows x 8 subtiles x 512)
NSUB = 8                # subtiles per super-tile
NT = NST * NSUB         # 64 (128-row) tiles covering all 8192 rows
PE_SUBS = (0, 1, 2)     # subtiles reduced on PE; the rest go to DVE
FP32 = mybir.dt.float32

_CACHE: dict = {}


def _build():
    nc = bacc.Bacc(
        "TRN2",
        target_bir_lowering=False,
        debug=False,
        num_devices=NCORES,
    )

    x_d = nc.dram_tensor("x", [B, FL], FP32, kind="ExternalInput")
    y_d = nc.dram_tensor("y", [B, FL], FP32, kind="ExternalInput")
    nz_d = nc.dram_tensor("noise", [BL, 1], FP32, kind="ExternalInput")
    out_d = nc.dram_tensor("out", [BL, 1], FP32, kind="ExternalOutput")

    y_r = y_d[:, :].rearrange("(s t p) f -> s p t f", p=128, t=NSUB)
    x_r = x_d[:, :].rearrange("(s t p) f -> s p t f", p=128, t=NSUB)
    nz_r = nz_d[:, 0].rearrange("(k p) -> p k", p=128)      # (128, 8)
    out_r = out_d[:, 0].rearrange("(k p) -> p k", p=128)    # (128, 8)

    with tile.TileContext(nc) as tc:
        with (
            tc.tile_pool(name="ypool", bufs=3) as ypool,
            tc.tile_pool(name="xpool", bufs=4) as xpool,
            tc.tile_pool(name="small", bufs=1) as small,
            tc.tile_pool(name="scratch", bufs=2) as scratch,
            tc.tile_pool(name="psum", bufs=1, space="PSUM") as psum,
            tc.tile_pool(name="dram", bufs=1, space="DRAM") as dram,
        ):
            ones_col = small.tile([128, 1], FP32)
            nc.gpsimd.memset(ones_col[:], 1.0)
            ones_row = small.tile([1, 128], FP32)
            nc.gpsimd.memset(ones_row[:], 1.0)

            # ---- phase A: ysum = sum over all 8192 rows of this core's
            # 512 y columns, accumulated while y streams in. PE takes 3
            # subtiles per super-tile (ones_col.T @ y_sub into a (1,512)
            # PSUM bank); DVE accumulates the other 5 into acc. ----
            ps_row = psum.tile([1, FL], FP32, tag="row")
            acc = small.tile([128, FL], FP32)
            first_dve = True
            xtiles = []
            for s in range(NST):
                ytile = ypool.tile([128, NSUB, FL], FP32, tag="y")
                # halves on the two HWDGE rings (sync + scalar) so both
                # DMA queues stream concurrently
                nc.sync.dma_start(ytile[:, 0:NSUB // 2, :],
                                  y_r[s, :, 0:NSUB // 2, :])
                nc.scalar.dma_start(ytile[:, NSUB // 2:, :],
                                    y_r[s, :, NSUB // 2:, :])
                for t in range(NSUB):
                    sub = ytile[:, t, :]
                    if t in PE_SUBS:
                        nc.tensor.matmul(
                            ps_row[0:1, :], ones_col[:], sub,
                            start=(s == 0 and t == 0), stop=False,
                        )
                    elif first_dve:
                        nc.vector.tensor_copy(acc[:], sub)
                        first_dve = False
                    else:
                        nc.vector.tensor_add(acc[:], acc[:], sub)
            # fold the DVE accumulator into the PSUM row (last of the
            # accumulation group)
            nc.tensor.matmul(ps_row[0:1, :], ones_col[:], acc[:],
                             start=False, stop=True)

            ysum_row = small.tile([1, FL], FP32)
            nc.vector.tensor_copy(ysum_row[:], ps_row[:])

            # ---- broadcast ysum across partitions: ones_row.T @ ysum ----
            bc = psum.tile([128, FL], FP32, tag="bc")
            nc.tensor.matmul(bc[:], ones_row[:], ysum_row[0:1, :],
                             start=True, stop=True)

            # ---- phase B: partial dots for ALL rows while x streams ----
            s_part = small.tile([128, NT], FP32)
            for s in range(NST):
                xtile = xpool.tile([128, NSUB, FL], FP32, tag="x")
                nc.sync.dma_start(xtile[:, 0:NSUB // 2, :],
                                  x_r[s, :, 0:NSUB // 2, :])
                nc.scalar.dma_start(xtile[:, NSUB // 2:, :],
                                    x_r[s, :, NSUB // 2:, :])
                for t in range(NSUB):
                    m = s * NSUB + t
                    prod = scratch.tile([128, FL], FP32, tag="sc")
                    nc.vector.scalar_tensor_tensor(
                        out=prod[:],
                        in0=xtile[:, t, :],
                        scalar=1.0,
                        in1=bc[:],
                        op0=mybir.AluOpType.mult,
                        op1=mybir.AluOpType.mult,
                        accum_out=s_part[:, m:m + 1],
                    )

            # ---- ReduceScatter: sum partials, keep our 1024-row shard ----
            cc_in = dram.tile([B], FP32)
            cc_out = dram.tile([BL], FP32)
            nc.gpsimd.dma_start(cc_in[:].rearrange("(t p) -> p t", p=128),
                                s_part[:])
            nc.gpsimd.collective_compute(
                "ReduceScatter",
                mybir.AluOpType.add,
                replica_groups=[list(range(NCORES))],
                ins=[cc_in.opt()],
                outs=[cc_out.opt()],
            )
            s_mine = small.tile([128, NSUB], FP32)
            nc.gpsimd.dma_start(s_mine[:],
                                cc_out[:].rearrange("(k p) -> p k", p=128))

            # ---- tail: hardswish, + noise, hardtanh ----
            noise_t = small.tile([128, NSUB], FP32)
            nc.gpsimd.dma_start(noise_t[:], nz_r)

            t_ = small.tile([128, NSUB], FP32)
            nc.vector.tensor_scalar(
                out=t_[:], in0=s_mine[:], scalar1=3.0, scalar2=0.0,
                op0=mybir.AluOpType.add, op1=mybir.AluOpType.max,
            )
            nc.vector.tensor_scalar(
                out=t_[:], in0=t_[:], scalar1=6.0, scalar2=1.0 / 6.0,
                op0=mybir.AluOpType.min, op1=mybir.AluOpType.mult,
            )
            r = small.tile([128, NSUB], FP32)
            nc.vector.tensor_tensor(
                out=r[:], in0=s_mine[:], in1=t_[:], op=mybir.AluOpType.mult,
            )
            nc.vector.tensor_tensor(
                out=r[:], in0=r[:], in1=noise_t[:], op=mybir.AluOpType.add,
            )
            nc.vector.tensor_scalar(
                out=r[:], in0=r[:], scalar1=-0.5, scalar2=0.5,
                op0=mybir.AluOpType.max, op1=mybir.AluOpType.min,
            )
            nc.gpsimd.dma_start(out_r, r[:])

    nc.compile()
    return nc


def _get_nc():
    if "nc" not in _CACHE:
        _CACHE["nc"] = _build()
    return _CACHE["nc"]


def kernel(x: np.ndarray, y: np.ndarray, noise: np.ndarray, **_run_kwargs) -> np.ndarray:
    x = np.ascontiguousarray(x, dtype=np.float32)
    y = np.ascontiguousarray(y, dtype=np.float32)
    noise = np.ascontiguousarray(noise, dtype=np.float32)

    nc = _get_nc()
    in_maps = [
        {
            "x": np.ascontiguousarray(x[:, i * FL:(i + 1) * FL]),
            "y": np.ascontiguousarray(y[:, i * FL:(i + 1) * FL]),
            "noise": noise[i * BL:(i + 1) * BL],
        }
        for i in range(NCORES)
    ]
    res = run_bass_kernel_spmd(nc, in_maps, list(range(NCORES)), **_run_kwargs)
    out = np.concatenate([res.results[i]["out"] for i in range(NCORES)], axis=0)
    if _run_kwargs:
        _CACHE["last_results"] = res
    return out


# revision 6
# speedup vs baseline: 1.1771x; 1.1771x over previous
"""Trainium2 Bass kernel for nn_Model_1580547969651.

Math (from the reference):
    s    = x @ sum(y, axis=0)          # (B,) row-sums of x @ y^T
    h    = hardswish(s)                # s * clip(s+3, 0, 6) / 6
    out  = clip(h + noise, -0.5, 0.5)  # (B, 1)

Strategy: COLUMN-shard x and y across the 8 cores (512 features each).
Each core's column-sum of its y shard is locally complete, so there is
no mid-kernel collective. While y streams in (split across both HWDGE
rings), the VectorEngine accumulates whole 2MB super-tiles; one strided
reduce folds the 8 subtiles and a single ones(128,128) matmul does the
partition-sum AND the 128-way broadcast in one shot. Phase B computes
partial dots s_i = x[:, F_i] @ ysum_i for ALL 8192 rows with fused
scalar_tensor_tensor while x streams. The partials are transposed on
the VectorEngine (32x32 blocks) so the ReduceScatter bounce DMA is
contiguous; the 32KB->4KB ReduceScatter hands each core its 1024-row
output shard, and the elementwise tail runs in a DMA-friendly (8,128)
layout.
"""

import numpy as np

from concourse import bass, bacc, mybir, tile
from concourse.bass_utils import run_bass_kernel_spmd

B = 8192
F = 4096
NCORES = 8
FL = F // NCORES        # 512 features per core
BL = B // NCORES        # 1024 output rows per core
NST = 8                 # y/x super-tiles (128 part x 8 subtiles x 512)
NSUB = 8                # subtiles per super-tile
NT = NST * NSUB         # 64 (128-row) tiles covering all 8192 rows
FP32 = mybir.dt.float32

_CACHE: dict = {}


def _build():
    nc = bacc.Bacc(
        "TRN2",
        target_bir_lowering=False,
        debug=False,
        num_devices=NCORES,
    )

    x_d = nc.dram_tensor("x", [B, FL], FP32, kind="ExternalInput")
    y_d = nc.dram_tensor("y", [B, FL], FP32, kind="ExternalInput")
    nz_d = nc.dram_tensor("noise", [BL, 1], FP32, kind="ExternalInput")
    out_d = nc.dram_tensor("out", [BL, 1], FP32, kind="ExternalOutput")

    y_r = y_d[:, :].rearrange("(s t p) f -> s p t f", p=128, t=NSUB)
    x_r = x_d[:, :].rearrange("(s t p) f -> s p t f", p=128, t=NSUB)
    nz_r = nz_d[:, 0].rearrange("(k p) -> k p", p=128)      # (8, 128) contig
    out_r = out_d[:, 0].rearrange("(k p) -> k p", p=128)    # (8, 128) contig

    with tile.TileContext(nc) as tc:
        with (
            tc.tile_pool(name="ypool", bufs=3) as ypool,
            tc.tile_pool(name="xpool", bufs=5) as xpool,
            tc.tile_pool(name="small", bufs=1) as small,
            tc.tile_pool(name="scratch", bufs=2) as scratch,
            tc.tile_pool(name="psum", bufs=1, space="PSUM") as psum,
            tc.tile_pool(name="dram", bufs=1, space="DRAM") as dram,
        ):
            ones128 = small.tile([128, 128], FP32)
            nc.gpsimd.memset(ones128[:], 1.0)

            # ---- phase A: accumulate y super-tiles on DVE ----
            acc = small.tile([128, NSUB, FL], FP32)
            for s in range(NST):
                ytile = ypool.tile([128, NSUB, FL], FP32, tag="y")
                nc.sync.dma_start(ytile[:, 0:NSUB // 2, :],
                                  y_r[s, :, 0:NSUB // 2, :])
                nc.scalar.dma_start(ytile[:, NSUB // 2:, :],
                                    y_r[s, :, NSUB // 2:, :])
                if s == 0:
                    nc.vector.tensor_copy(acc[:], ytile[:])
                else:
                    nc.vector.tensor_add(acc[:], acc[:], ytile[:])
            # fold the 8 subtiles: (128, 8, 512) -> (128, 512)
            ysum128 = small.tile([128, FL], FP32)
            nc.vector.tensor_reduce(
                out=ysum128[:],
                in_=acc[:].rearrange("p t f -> p f t"),
                axis=mybir.AxisListType.X,
                op=mybir.AluOpType.add,
            )
            # partition-sum + 128-way broadcast in ONE matmul:
            # bc[q, f] = sum_p ones[p, q] * ysum128[p, f]
            bc = psum.tile([128, FL], FP32, tag="bc")
            nc.tensor.matmul(bc[:], ones128[:], ysum128[:],
                             start=True, stop=True)

            # ---- phase B: partial dots for ALL rows while x streams ----
            s_part = small.tile([128, NT], FP32)
            for s in range(NST):
                xtile = xpool.tile([128, NSUB, FL], FP32, tag="x")
                nc.sync.dma_start(xtile[:, 0:NSUB // 2, :],
                                  x_r[s, :, 0:NSUB // 2, :])
                nc.scalar.dma_start(xtile[:, NSUB // 2:, :],
                                    x_r[s, :, NSUB // 2:, :])
                for t in range(NSUB):
                    m = s * NSUB + t
                    prod = scratch.tile([128, FL], FP32, tag="sc")
                    nc.vector.scalar_tensor_tensor(
                        out=prod[:],
                        in0=xtile[:, t, :],
                        scalar=1.0,
                        in1=bc[:],
                        op0=mybir.AluOpType.mult,
                        op1=mybir.AluOpType.mult,
                        accum_out=s_part[:, m:m + 1],
                    )

            # ---- transpose s_part (128, 64) -> (64, 128) in 32x32 blocks
            # so the ReduceScatter bounce buffer is written contiguously ----
            s_t = small.tile([64, 128], FP32)
            for i in range(4):
                for j in range(2):
                    nc.vector.transpose(
                        s_t[32 * j:32 * (j + 1), 32 * i:32 * (i + 1)],
                        s_part[32 * i:32 * (i + 1), 32 * j:32 * (j + 1)],
                    )

            # ---- ReduceScatter: sum partials, keep our 1024-row shard ----
            cc_in = dram.tile([B], FP32)
            cc_out = dram.tile([BL], FP32)
            nc.gpsimd.dma_start(cc_in[:].rearrange("(m p) -> m p", p=128),
                                s_t[:])
            nc.gpsimd.collective_compute(
                "ReduceScatter",
                mybir.AluOpType.add,
                replica_groups=[list(range(NCORES))],
                ins=[cc_in.opt()],
                outs=[cc_out.opt()],
            )
            s_mine = small.tile([NSUB, 128], FP32)
            nc.gpsimd.dma_start(s_mine[:],
                                cc_out[:].rearrange("(k p) -> k p", p=128))

            # ---- tail: hardswish, + noise, hardtanh (in (8,128) layout) ----
            noise_t = small.tile([NSUB, 128], FP32)
            nc.gpsimd.dma_start(noise_t[:], nz_r)

            t_ = small.tile([NSUB, 128], FP32)
            nc.vector.tensor_scalar(
                out=t_[:], in0=s_mine[:], scalar1=3.0, scalar2=0.0,
                op0=mybir.AluOpType.add, op1=mybir.AluOpType.max,
            )
            nc.vector.tensor_scalar(
                out=t_[:], in0=t_[:], scalar1=6.0, scalar2=1.0 / 6.0,
                op0=mybir.AluOpType.min, op1=mybir.AluOpType.mult,
            )
            r = small.tile([NSUB, 128], FP32)
            nc.vector.tensor_tensor(
                out=r[:], in0=s_mine[:], in1=t_[:], op=mybir.AluOpType.mult,
            )
            nc.vector.tensor_tensor(
                out=r[:], in0=r[:], in1=noise_t[:], op=mybir.AluOpType.add,
            )
            nc.vector.tensor_scalar(
                out=r[:], in0=r[:], scalar1=-0.5, scalar2=0.5,
                op0=mybir.AluOpType.max, op1=mybir.AluOpType.min,
            )
            nc.gpsimd.dma_start(out_r, r[:])

    nc.compile()
    return nc


def _get_nc():
    if "nc" not in _CACHE:
        _CACHE["nc"] = _build()
    return _CACHE["nc"]


def kernel(x: np.ndarray, y: np.ndarray, noise: np.ndarray, **_run_kwargs) -> np.ndarray:
    x = np.ascontiguousarray(x, dtype=np.float32)
    y = np.ascontiguousarray(y, dtype=np.float32)
    noise = np.ascontiguousarray(noise, dtype=np.float32)

    nc = _get_nc()
    in_maps = [
        {
            "x": np.ascontiguousarray(x[:, i * FL:(i + 1) * FL]),
            "y": np.ascontiguousarray(y[:, i * FL:(i + 1) * FL]),
            "noise": noise[i * BL:(i + 1) * BL],
        }
        for i in range(NCORES)
    ]
    res = run_bass_kernel_spmd(nc, in_maps, list(range(NCORES)), **_run_kwargs)
    out = np.concatenate([res.results[i]["out"] for i in range(NCORES)], axis=0)
    if _run_kwargs:
        _CACHE["last_results"] = res
    return out


# revision 8
# speedup vs baseline: 1.2092x; 1.0272x over previous
"""Trainium2 Bass kernel for nn_Model_1580547969651.

Math (from the reference):
    s    = x @ sum(y, axis=0)          # (B,) row-sums of x @ y^T
    h    = hardswish(s)                # s * clip(s+3, 0, 6) / 6
    out  = clip(h + noise, -0.5, 0.5)  # (B, 1)

Strategy: COLUMN-shard x and y across the 8 cores (512 features each).
Each core's column-sum of its y shard is locally complete, so there is
no mid-kernel collective. While y streams in (split across both HWDGE
rings), the VectorEngine accumulates whole 2MB super-tiles; one strided
reduce folds the 8 subtiles and a single ones(128,128) matmul does the
partition-sum AND the 128-way broadcast in one shot. Phase B computes
partial dots s_i = x[:, F_i] @ ysum_i for ALL 8192 rows with fused
scalar_tensor_tensor while x streams. The partials are transposed on
the VectorEngine (32x32 blocks) so the ReduceScatter bounce DMA is
contiguous; the 32KB->4KB ReduceScatter hands each core its 1024-row
output shard, and the elementwise tail runs in a DMA-friendly (8,128)
layout.
"""

import numpy as np

from concourse import bass, bacc, mybir, tile
from concourse.bass_utils import run_bass_kernel_spmd

B = 8192
F = 4096
NCORES = 8
FL = F // NCORES        # 512 features per core
BL = B // NCORES        # 1024 output rows per core
NST = 8                 # y/x super-tiles (128 part x 8 subtiles x 512)
NSUB = 8                # subtiles per super-tile
NT = NST * NSUB         # 64 (128-row) tiles covering all 8192 rows
FP32 = mybir.dt.float32

_CACHE: dict = {}


def _build():
    nc = bacc.Bacc(
        "TRN2",
        target_bir_lowering=False,
        debug=False,
        num_devices=NCORES,
    )

    x_d = nc.dram_tensor("x", [B, FL], FP32, kind="ExternalInput")
    y_d = nc.dram_tensor("y", [B, FL], FP32, kind="ExternalInput")
    nz_d = nc.dram_tensor("noise", [BL, 1], FP32, kind="ExternalInput")
    out_d = nc.dram_tensor("out", [BL, 1], FP32, kind="ExternalOutput")

    y_r = y_d[:, :].rearrange("(s t p) f -> s p t f", p=128, t=NSUB)
    x_r = x_d[:, :].rearrange("(s t p) f -> s p t f", p=128, t=NSUB)
    nz_r = nz_d[:, 0].rearrange("(k p) -> k p", p=128)      # (8, 128) contig
    out_r = out_d[:, 0].rearrange("(k p) -> k p", p=128)    # (8, 128) contig

    with tile.TileContext(nc) as tc:
        with (
            tc.tile_pool(name="ypool", bufs=5) as ypool,
            tc.tile_pool(name="xpool", bufs=4) as xpool,
            tc.tile_pool(name="small", bufs=1) as small,
            tc.tile_pool(name="scratch", bufs=2) as scratch,
            tc.tile_pool(name="psum", bufs=1, space="PSUM") as psum,
            tc.tile_pool(name="dram", bufs=1, space="DRAM") as dram,
        ):
            ones128 = small.tile([128, 128], FP32)
            nc.gpsimd.memset(ones128[:], 1.0)

            # ---- phase A: accumulate y super-tiles on DVE ----
            acc = small.tile([128, NSUB, FL], FP32)
            for s in range(NST):
                ytile = ypool.tile([128, NSUB, FL], FP32, tag="y")
                nc.sync.dma_start(ytile[:, 0:NSUB // 2, :],
                                  y_r[s, :, 0:NSUB // 2, :])
                nc.scalar.dma_start(ytile[:, NSUB // 2:, :],
                                    y_r[s, :, NSUB // 2:, :])
                if s == 0:
                    nc.vector.tensor_copy(acc[:], ytile[:])
                else:
                    nc.vector.tensor_add(acc[:], acc[:], ytile[:])
            # fold the 8 subtiles with a contiguous pairwise tree:
            # (128, 8, 512) -> (128, 4, 512) -> (128, 2, 512) -> (128, 512)
            nc.vector.tensor_add(acc[:, 0:4, :], acc[:, 0:4, :], acc[:, 4:8, :])
            nc.vector.tensor_add(acc[:, 0:2, :], acc[:, 0:2, :], acc[:, 2:4, :])
            ysum128 = small.tile([128, FL], FP32)
            nc.vector.tensor_tensor(
                out=ysum128[:], in0=acc[:, 0, :], in1=acc[:, 1, :],
                op=mybir.AluOpType.add,
            )
            # partition-sum + 128-way broadcast in ONE matmul:
            # bc[q, f] = sum_p ones[p, q] * ysum128[p, f]
            bc = psum.tile([128, FL], FP32, tag="bc")
            nc.tensor.matmul(bc[:], ones128[:], ysum128[:],
                             start=True, stop=True)

            # ---- phase B: partial dots for ALL rows while x streams ----
            s_part = small.tile([128, NT], FP32)
            for s in range(NST):
                xtile = xpool.tile([128, NSUB, FL], FP32, tag="x")
                nc.sync.dma_start(xtile[:, 0:NSUB // 2, :],
                                  x_r[s, :, 0:NSUB // 2, :])
                nc.scalar.dma_start(xtile[:, NSUB // 2:, :],
                                    x_r[s, :, NSUB // 2:, :])
                for t in range(NSUB):
                    m = s * NSUB + t
                    prod = scratch.tile([128, FL], FP32, tag="sc")
                    nc.vector.scalar_tensor_tensor(
                        out=prod[:],
                        in0=xtile[:, t, :],
                        scalar=1.0,
                        in1=bc[:],
                        op0=mybir.AluOpType.mult,
                        op1=mybir.AluOpType.mult,
                        accum_out=s_part[:, m:m + 1],
                    )

            # ---- transpose s_part (128, 64) -> (64, 128) in 32x32 blocks
            # so the ReduceScatter bounce buffer is written contiguously ----
            s_t = small.tile([64, 128], FP32)
            for i in range(4):
                for j in range(2):
                    nc.vector.transpose(
                        s_t[32 * j:32 * (j + 1), 32 * i:32 * (i + 1)],
                        s_part[32 * i:32 * (i + 1), 32 * j:32 * (j + 1)],
                    )

            # ---- ReduceScatter: sum partials, keep our 1024-row shard ----
            cc_in = dram.tile([B], FP32)
            cc_out = dram.tile([BL], FP32)
            nc.gpsimd.dma_start(cc_in[:].rearrange("(m p) -> m p", p=128),
                                s_t[:])
            nc.gpsimd.collective_compute(
                "ReduceScatter",
                mybir.AluOpType.add,
                replica_groups=[list(range(NCORES))],
                ins=[cc_in.opt()],
                outs=[cc_out.opt()],
            )
            s_mine = small.tile([NSUB, 128], FP32)
            nc.gpsimd.dma_start(s_mine[:],
                                cc_out[:].rearrange("(k p) -> k p", p=128))

            # ---- tail: hardswish, + noise, hardtanh (in (8,128) layout) ----
            noise_t = small.tile([NSUB, 128], FP32)
            nc.gpsimd.dma_start(noise_t[:], nz_r)

            t_ = small.tile([NSUB, 128], FP32)
            nc.vector.tensor_scalar(
                out=t_[:], in0=s_mine[:], scalar1=3.0, scalar2=0.0,
                op0=mybir.AluOpType.add, op1=mybir.AluOpType.max,
            )
            nc.vector.tensor_scalar(
                out=t_[:], in0=t_[:], scalar1=6.0, scalar2=1.0 / 6.0,
                op0=mybir.AluOpType.min, op1=mybir.AluOpType.mult,
            )
            r = small.tile([NSUB, 128], FP32)
            nc.vector.tensor_tensor(
                out=r[:], in0=s_mine[:], in1=t_[:], op=mybir.AluOpType.mult,
            )
            nc.vector.tensor_tensor(
                out=r[:], in0=r[:], in1=noise_t[:], op=mybir.AluOpType.add,
            )
            nc.vector.tensor_scalar(
                out=r[:], in0=r[:], scalar1=-0.5, scalar2=0.5,
                op0=mybir.AluOpType.max, op1=mybir.AluOpType.min,
            )
            nc.gpsimd.dma_start(out_r, r[:])

    nc.compile()
    return nc


def _get_nc():
    if "nc" not in _CACHE:
        _CACHE["nc"] = _build()
    return _CACHE["nc"]


def kernel(x: np.ndarray, y: np.ndarray, noise: np.ndarray, **_run_kwargs) -> np.ndarray:
    x = np.ascontiguousarray(x, dtype=np.float32)
    y = np.ascontiguousarray(y, dtype=np.float32)
    noise = np.ascontiguousarray(noise, dtype=np.float32)

    nc = _get_nc()
    in_maps = [
        {
            "x": np.ascontiguousarray(x[:, i * FL:(i + 1) * FL]),
            "y": np.ascontiguousarray(y[:, i * FL:(i + 1) * FL]),
            "noise": noise[i * BL:(i + 1) * BL],
        }
        for i in range(NCORES)
    ]
    res = run_bass_kernel_spmd(nc, in_maps, list(range(NCORES)), **_run_kwargs)
    out = np.concatenate([res.results[i]["out"] for i in range(NCORES)], axis=0)
    if _run_kwargs:
        _CACHE["last_results"] = res
    return out
